# revision 1
# baseline (speedup 1.0000x reference)
"""EntailmentConeLoss on 8 Trainium2 NeuronCores.

Strategy (data-parallel over the pair dim, sharding_hint):
- Each core takes P/8 = 8192 pairs and their 4*8192 negatives; the prototype
  table is replicated.
- Row gathers use gpsimd dma_gather. Its indices are int16, so rows are
  addressed via the mod-4 residue trick: the [100000, 256] f32 table is viewed
  with a 4-row stride (4096B <= the 255*256B stride limit), giving quotient
  indices < 25000 that fit in int16. Pairs/negatives are bucket-sorted on the
  host by (class(parent), class(child)) so every gather call is class-pure.
- Per gathered row pair (a, c): pp = <a,a>, cc = <c,c>, pc = <a,c> via fused
  DVE tensor_tensor_reduce and ACT Square+accum (split across engines for
  balance), then a wide vectorized epilogue computes the cone energy:
    dd  = relu(cc - 2 pc + pp)
    den = 2*sqrt(pp*dd) + 1e-6          (ACT sqrt + one Newton step)
    cos = clip(2 (pc - pp) / den)
    ang = pi/2 - arctan(cos / sqrt(1 - cos^2))
    ap  = asin_small(0.1 / (sqrt(pp) + 1e-6))
    e   = relu(ang - ap)
- Bucket padding slots gather row 0 of the class and are masked out of the
  partial sums. Per-core partial sums [128, 2] are combined on the host.
"""
import os
os.environ.setdefault("NEURON_RT_RESET_CORES", "1")

import numpy as np

C, D = 100000, 256
P_TOT, K = 65536, 4
NCORES = 8
PPC = P_TOT // NCORES          # pairs per core
NPC = PPC * K                  # negatives per core
NBUCK = 16                     # (class_a, class_c) buckets
EPS = np.float32(1e-6)
BETA = np.float32(0.1)
MARGIN = np.float32(0.1)

_CACHE = {}


def _build_program(capp, capn, loop_iters=1, stage=2):
    import concourse.bass as bass
    import concourse.bacc as bacc
    import concourse.mybir as mybir
    import concourse.tile as tile

    f32 = mybir.dt.float32
    i16 = mybir.dt.int16
    Alu = mybir.AluOpType
    Act = mybir.ActivationFunctionType

    bp = capp // 128           # pos slot-columns per bucket
    bn = capn // 128           # neg slot-columns per bucket
    NPCOL = NBUCK * bp
    NNCOL = NBUCK * bn

    nc = bacc.Bacc("TRN2", target_bir_lowering=False, num_devices=NCORES, num_swdge_queues=4)
    table = nc.dram_tensor("prototypes", [C, D], f32, kind="ExternalInput")
    posa_i = nc.dram_tensor("posa_i", [128, NPCOL * 8], i16, kind="ExternalInput")
    posb_i = nc.dram_tensor("posb_i", [128, NPCOL * 8], i16, kind="ExternalInput")
    nega_i = nc.dram_tensor("nega_i", [128, NNCOL * 8], i16, kind="ExternalInput")
    negc_i = nc.dram_tensor("negc_i", [128, NNCOL * 8], i16, kind="ExternalInput")
    maskp = nc.dram_tensor("maskp", [128, NPCOL], f32, kind="ExternalInput")
    maskn = nc.dram_tensor("maskn", [128, NNCOL], f32, kind="ExternalInput")
    partials = nc.dram_tensor("partials", [128, 2], f32, kind="ExternalOutput")

    HALF_PI = float(np.float32(np.pi / 2))

    with tile.TileContext(nc) as tc:
        with tc.tile_pool(name="io", bufs=1) as io, \
             tc.tile_pool(name="prow", bufs=3) as prow, \
             tc.tile_pool(name="sq", bufs=2) as sqp, \
             tc.tile_pool(name="tmp", bufs=1) as tmp:

            posa_t = io.tile([128, NPCOL * 8], i16)
            posb_t = io.tile([128, NPCOL * 8], i16)
            nega_t = io.tile([128, NNCOL * 8], i16)
            negc_t = io.tile([128, NNCOL * 8], i16)
            maskp_t = io.tile([128, NPCOL], f32)
            maskn_t = io.tile([128, NNCOL], f32)
            nc.sync.dma_start(out=posa_t[:], in_=posa_i[:])
            nc.sync.dma_start(out=posb_t[:], in_=posb_i[:])
            nc.sync.dma_start(out=nega_t[:], in_=nega_i[:])
            nc.sync.dma_start(out=negc_t[:], in_=negc_i[:])
            nc.sync.dma_start(out=maskp_t[:], in_=maskp[:])
            nc.sync.dma_start(out=maskn_t[:], in_=maskn[:])

            # accumulator buffers
            pp_p = io.tile([128, NPCOL], f32)
            cc_p = io.tile([128, NPCOL], f32)
            pc_p = io.tile([128, NPCOL], f32)
            pp_n = io.tile([128, NNCOL], f32)
            cc_n = io.tile([128, NNCOL], f32)
            pc_n = io.tile([128, NNCOL], f32)
            out_t = io.tile([128, 2], f32)

            tview = table[:].rearrange("(q r) d -> q r d", r=4)

            CCH = 8    # slot-columns per gather+compute chunk
            FSPLIT_A = 7   # every FSPLIT_A-th a-column uses the split route

            col_counter = [0]
            qrr = [0]

            def loop_body(_i=None):
                def do_chunk(colbase, w, ca, cb, a_idx_t, c_idx_t,
                             pp_b, cc_b, pc_b):
                    a_rows = prow.tile([128, CCH, D], f32, tag="ra", name="ra")
                    c_rows = prow.tile([128, CCH, D], f32, tag="rc", name="rc")
                    nc.gpsimd.dma_gather(
                        a_rows[:, :w, :], tview[:, ca, :],
                        a_idx_t[:, colbase * 8:(colbase + w) * 8],
                        w * 128, w * 128, D, elem_step=4 * D,
                        single_packet=False, queue_num=qrr[0] % 4)
                    nc.gpsimd.dma_gather(
                        c_rows[:, :w, :], tview[:, cb, :],
                        c_idx_t[:, colbase * 8:(colbase + w) * 8],
                        w * 128, w * 128, D, elem_step=4 * D,
                        single_packet=False, queue_num=(qrr[0] + 1) % 4)
                    qrr[0] += 2
                    if stage < 1:
                        return
                    # a-stream squares: mostly ACT fused square+accum
                    act_scr = sqp.tile([128, D], f32, tag="ascr", name="ascr")
                    wsq = sqp.tile([128, CCH, D], f32, tag="wsq", name="wsq")
                    for c in range(w):
                        col_counter[0] += 1
                        if col_counter[0] % FSPLIT_A == 0:
                            nc.scalar.activation(
                                out=wsq[:, c, :], in_=a_rows[:, c, :],
                                func=Act.Square)
                            nc.vector.tensor_scalar(
                                out=wsq[:, c, :], in0=wsq[:, c, :],
                                scalar1=1.0, scalar2=0.0,
                                op0=Alu.mult, op1=Alu.add,
                                accum_out=pp_b[:, colbase + c:colbase + c + 1])
                        else:
                            nc.scalar.activation(
                                out=act_scr[:], in_=a_rows[:, c, :], func=Act.Square,
                                accum_out=pp_b[:, colbase + c:colbase + c + 1])
                    # c-stream squares: wide ACT square + per-col DVE ts-accum
                    csq = sqp.tile([128, CCH, D], f32, tag="csq", name="csq")
                    nc.scalar.activation(
                        out=csq[:, :w, :].rearrange("p a b -> p (a b)"),
                        in_=c_rows[:, :w, :].rearrange("p a b -> p (a b)"),
                        func=Act.Square)
                    for c in range(w):
                        nc.vector.tensor_scalar(
                            out=csq[:, c, :], in0=csq[:, c, :],
                            scalar1=1.0, scalar2=0.0,
                            op0=Alu.mult, op1=Alu.add,
                            accum_out=cc_b[:, colbase + c:colbase + c + 1])
                    # pc: wide DVE mult + per-col DVE ts-accum
                    wpr = sqp.tile([128, CCH, D], f32, tag="wpr", name="wpr")
                    nc.vector.tensor_tensor(
                        out=wpr[:, :w, :].rearrange("p a b -> p (a b)"),
                        in0=a_rows[:, :w, :].rearrange("p a b -> p (a b)"),
                        in1=c_rows[:, :w, :].rearrange("p a b -> p (a b)"),
                        op=Alu.mult)
                    for c in range(w):
                        nc.vector.tensor_scalar(
                            out=wpr[:, c, :], in0=wpr[:, c, :],
                            scalar1=1.0, scalar2=0.0,
                            op0=Alu.mult, op1=Alu.add,
                            accum_out=pc_b[:, colbase + c:colbase + c + 1])

                def do_bucket(xy, nb, a_idx_t, c_idx_t, pp_b, cc_b, pc_b, ca, cb):
                    c0 = xy * nb
                    for s in range(0, nb, CCH):
                        w = min(CCH, nb - s)
                        do_chunk(c0 + s, w, ca, cb, a_idx_t, c_idx_t,
                                 pp_b, cc_b, pc_b)

                for xy in range(NBUCK):
                    do_bucket(xy, bp, posa_t, posb_t, pp_p, cc_p, pc_p,
                              xy // 4, xy % 4)
                for xy in range(NBUCK):
                    do_bucket(xy, bn, nega_t, negc_t, pp_n, cc_n, pc_n,
                              xy // 4, xy % 4)

                # ---------------- epilogue (wide ops) ----------------
                def epilogue(pp_b, cc_b, pc_b, mask_t, ncol, tagn, is_neg, out_col):
                    # shared tags across P/N epilogues (sequential) to save SBUF
                    T = lambda nm: tmp.tile([128, ncol], f32, tag="ep" + nm,
                                            name="ep" + nm)
                    ppcc = T("ppcc")
                    nc.vector.tensor_tensor(out=ppcc[:], in0=cc_b[:], in1=pp_b[:], op=Alu.add)
                    t2 = T("t2")
                    nc.vector.tensor_scalar(out=t2[:], in0=pc_b[:], scalar1=-2.0,
                                            scalar2=None, op0=Alu.mult)
                    dd = T("dd")
                    nc.vector.tensor_tensor(out=dd[:], in0=ppcc[:], in1=t2[:], op=Alu.add)
                    # duplicate-row guard: true dd is >= 0; for c==p the three
                    # dot products come from different engine paths, so dd is
                    # rounding junk ~1e-4 instead of exactly 0. Flag dd below
                    # 1e-5*(pp+cc) and force cos=0 (-> ang=pi/2) as the
                    # reference's eps-denominator path does.
                    dupf = T("dupf")
                    nc.vector.tensor_scalar(out=dupf[:], in0=ppcc[:], scalar1=1e-5,
                                            scalar2=None, op0=Alu.mult)
                    nc.vector.tensor_tensor(out=dupf[:], in0=dd[:], in1=dupf[:], op=Alu.is_lt)
                    nc.vector.tensor_scalar(out=dupf[:], in0=dupf[:], scalar1=-1.0,
                                            scalar2=1.0, op0=Alu.mult, op1=Alu.add)
                    nc.vector.tensor_scalar(out=dd[:], in0=dd[:], scalar1=0.0,
                                            scalar2=None, op0=Alu.max)
                    g = T("g")
                    nc.vector.tensor_tensor(out=g[:], in0=pp_b[:], in1=dd[:], op=Alu.mult)
                    # s = sqrt(g) refined; guard zeros via +tiny trick instead of select:
                    # g==0 -> s0=0 -> r=inf -> nan. Add tiny 1e-30: sqrt ~ 1e-15,
                    # refined fine, den ~ 1e-6 dominated by eps. cos = 0/eps = 0. OK.
                    nc.vector.tensor_scalar(out=g[:], in0=g[:], scalar1=1e-30,
                                            scalar2=None, op0=Alu.add)
                    s0 = T("s0")
                    nc.scalar.activation(out=s0[:], in_=g[:], func=Act.Sqrt)
                    r = T("r")
                    nc.vector.reciprocal(r[:], s0[:])
                    s1 = T("s1")
                    nc.vector.tensor_tensor(out=s1[:], in0=g[:], in1=r[:], op=Alu.mult)
                    nc.vector.tensor_tensor(out=s1[:], in0=s1[:], in1=s0[:], op=Alu.add)
                    # den = 0.5*s1*2 + eps = s1 + eps  (0.5 and 2 cancel)
                    den = T("den")
                    nc.vector.tensor_scalar(out=den[:], in0=s1[:], scalar1=float(EPS),
                                            scalar2=None, op0=Alu.add)
                    rden = T("rden")
                    nc.vector.reciprocal(rden[:], den[:])
                    num = T("num")
                    nc.vector.tensor_tensor(out=num[:], in0=pc_b[:], in1=pp_b[:], op=Alu.subtract)
                    cos = T("cos")
                    nc.vector.tensor_tensor(out=cos[:], in0=num[:], in1=rden[:], op=Alu.mult)
                    nc.vector.tensor_scalar(out=cos[:], in0=cos[:], scalar1=2.0,
                                            scalar2=float(-(1.0 - 1e-6)), op0=Alu.mult,
                                            op1=Alu.max)
                    nc.vector.tensor_scalar(out=cos[:], in0=cos[:], scalar1=float(1.0 - 1e-6),
                                            scalar2=None, op0=Alu.min)
                    nc.vector.tensor_tensor(out=cos[:], in0=cos[:], in1=dupf[:], op=Alu.mult)
                    # ang = arccos(cos) via octant-reduced arctan:
                    #   s = sqrt(1-cos^2); r = min(|cos|,s)/max(|cos|,s) in [0,1]
                    #   |cos|<=s: ang = pi/2 - sign(cos)*arctan(r)
                    #   cos >  s: ang = arctan(r);  cos < -s: ang = pi - arctan(r)
                    q = T("q")
                    nc.vector.tensor_tensor(out=q[:], in0=cos[:], in1=cos[:], op=Alu.mult)
                    nc.vector.tensor_scalar(out=q[:], in0=q[:], scalar1=-1.0,
                                            scalar2=1.0, op0=Alu.mult, op1=Alu.add)
                    # q >= 1 - (1-1e-6)^2 ~ 2e-6 > 0 (clip guarantees)
                    q0 = T("q0")
                    nc.scalar.activation(out=q0[:], in_=q[:], func=Act.Sqrt)
                    rq = T("rq")
                    nc.vector.reciprocal(rq[:], q0[:])
                    sq = T("sq")
                    nc.vector.tensor_tensor(out=sq[:], in0=q[:], in1=rq[:], op=Alu.mult)
                    nc.vector.tensor_tensor(out=sq[:], in0=sq[:], in1=q0[:], op=Alu.add)
                    nc.vector.tensor_scalar(out=sq[:], in0=sq[:], scalar1=0.5,
                                            scalar2=None, op0=Alu.mult)
                    abst = T("abst")
                    nc.vector.tensor_scalar(out=abst[:], in0=cos[:], scalar1=-1.0,
                                            scalar2=None, op0=Alu.mult)
                    nc.vector.tensor_tensor(out=abst[:], in0=abst[:], in1=cos[:], op=Alu.max)
                    u = T("u")
                    nc.vector.tensor_tensor(out=u[:], in0=abst[:], in1=sq[:], op=Alu.min)
                    v = T("v")
                    nc.vector.tensor_tensor(out=v[:], in0=abst[:], in1=sq[:], op=Alu.max)
                    rv = T("rv")
                    nc.vector.reciprocal(rv[:], v[:])
                    rr = T("rr")
                    nc.vector.tensor_tensor(out=rr[:], in0=u[:], in1=rv[:], op=Alu.mult)
                    at = T("at")
                    nc.scalar.activation(out=at[:], in_=rr[:], func=Act.Arctan)
                    pg = T("pg")
                    nc.vector.tensor_scalar(out=pg[:], in0=cos[:], scalar1=0.0,
                                            scalar2=None, op0=Alu.is_gt)
                    ng = T("ng")
                    nc.vector.tensor_scalar(out=ng[:], in0=cos[:], scalar1=0.0,
                                            scalar2=None, op0=Alu.is_lt)
                    sgn = T("sgn")
                    nc.vector.tensor_tensor(out=sgn[:], in0=pg[:], in1=ng[:], op=Alu.subtract)
                    big = T("big")
                    nc.vector.tensor_tensor(out=big[:], in0=abst[:], in1=sq[:], op=Alu.is_gt)
                    c1 = T("c1")
                    nc.vector.tensor_scalar(out=c1[:], in0=big[:], scalar1=2.0,
                                            scalar2=-1.0, op0=Alu.mult, op1=Alu.add)
                    nc.vector.tensor_tensor(out=c1[:], in0=c1[:], in1=sgn[:], op=Alu.mult)
                    c0 = T("c0")
                    nc.vector.tensor_tensor(out=c0[:], in0=big[:], in1=ng[:], op=Alu.mult)
                    nc.vector.tensor_scalar(out=c0[:], in0=c0[:], scalar1=float(np.pi),
                                            scalar2=None, op0=Alu.mult)
                    c0b = T("c0b")
                    nc.vector.tensor_scalar(out=c0b[:], in0=big[:], scalar1=-HALF_PI,
                                            scalar2=HALF_PI, op0=Alu.mult, op1=Alu.add)
                    nc.vector.tensor_tensor(out=c0[:], in0=c0[:], in1=c0b[:], op=Alu.add)
                    ang = T("ang")
                    nc.vector.tensor_tensor(out=ang[:], in0=c1[:], in1=at[:], op=Alu.mult)
                    nc.vector.tensor_tensor(out=ang[:], in0=ang[:], in1=c0[:], op=Alu.add)
                    # aperture
                    sp0 = T("sp0")
                    nc.scalar.activation(out=sp0[:], in_=pp_b[:], func=Act.Sqrt)
                    rp = T("rp")
                    nc.vector.reciprocal(rp[:], sp0[:])
                    sp1 = T("sp1")
                    nc.vector.tensor_tensor(out=sp1[:], in0=pp_b[:], in1=rp[:], op=Alu.mult)
                    nc.vector.tensor_tensor(out=sp1[:], in0=sp1[:], in1=sp0[:], op=Alu.add)
                    nc.vector.tensor_scalar(out=sp1[:], in0=sp1[:], scalar1=0.5,
                                            scalar2=float(EPS), op0=Alu.mult, op1=Alu.add)
                    rsp = T("rsp")
                    nc.vector.reciprocal(rsp[:], sp1[:])
                    y = T("y")
                    nc.vector.tensor_scalar(out=y[:], in0=rsp[:], scalar1=float(BETA),
                                            scalar2=0.0, op0=Alu.mult, op1=Alu.max)
                    nc.vector.tensor_scalar(out=y[:], in0=y[:], scalar1=float(1.0 - 1e-6),
                                            scalar2=None, op0=Alu.min)
                    y2 = T("y2")
                    nc.vector.tensor_tensor(out=y2[:], in0=y[:], in1=y[:], op=Alu.mult)
                    y3 = T("y3")
                    nc.vector.tensor_tensor(out=y3[:], in0=y2[:], in1=y[:], op=Alu.mult)
                    ap = T("ap")
                    nc.vector.tensor_scalar(out=ap[:], in0=y3[:], scalar1=float(1.0 / 6.0),
                                            scalar2=None, op0=Alu.mult)
                    nc.vector.tensor_tensor(out=ap[:], in0=ap[:], in1=y[:], op=Alu.add)
                    e = T("e")
                    nc.vector.tensor_tensor(out=e[:], in0=ang[:], in1=ap[:], op=Alu.subtract)
                    nc.vector.tensor_scalar(out=e[:], in0=e[:], scalar1=0.0,
                                            scalar2=None, op0=Alu.max)
                    if is_neg:
                        nc.vector.tensor_scalar(out=e[:], in0=e[:], scalar1=-1.0,
                                                scalar2=float(MARGIN), op0=Alu.mult,
                                                op1=Alu.add)
                        nc.vector.tensor_scalar(out=e[:], in0=e[:], scalar1=0.0,
                                                scalar2=None, op0=Alu.max)
                    nc.vector.tensor_tensor(out=e[:], in0=e[:], in1=mask_t[:], op=Alu.mult)
                    nc.vector.tensor_reduce(
                        out=out_t[:, out_col:out_col + 1], in_=e[:],
                        axis=mybir.AxisListType.X, op=Alu.add)

                if stage == 2:
                    epilogue(pp_p, cc_p, pc_p, maskp_t, NPCOL, "P", False, 0)
                    epilogue(pp_n, cc_n, pc_n, maskn_t, NNCOL, "N", True, 1)
                else:
                    nc.vector.tensor_reduce(out=out_t[:, 0:1], in_=maskp_t[:],
                                            axis=mybir.AxisListType.X, op=Alu.add)
                    nc.vector.tensor_reduce(out=out_t[:, 1:2], in_=maskn_t[:],
                                            axis=mybir.AxisListType.X, op=Alu.add)
                nc.sync.dma_start(out=partials[:], in_=out_t[:])

            if loop_iters > 1:
                with tc.For_i(0, loop_iters, 1):
                    loop_body()
            else:
                loop_body()

    nc.compile()
    return nc


def _wrap_idx(q, cap):
    """[cap] int16 -> [128, cap//16] wrapped+replicated layout."""
    w = q.reshape(cap // 16, 16).T           # [16, cap//16]
    return np.tile(w, (8, 1))                # [128, cap//16]


def _prep_stream(a_vals, c_vals, cap):
    """Bucket-sort (a, c) index pairs by (a%4, c%4); pad each bucket to cap.

    Returns wrapped int16 a-idx, c-idx [128, 16*cap//16], mask [128, ncol]."""
    n = a_vals.shape[0]
    key = (a_vals % 4) * 4 + (c_vals % 4)
    order = np.argsort(key, kind="stable")
    counts = np.bincount(key, minlength=NBUCK)
    assert counts.max() <= cap, (counts.max(), cap)
    a_q = np.zeros(NBUCK * cap, np.int16)
    c_q = np.zeros(NBUCK * cap, np.int16)
    mask = np.zeros(NBUCK * cap, np.float32)
    off_src = 0
    for xy in range(NBUCK):
        cnt = int(counts[xy])
        seg = order[off_src:off_src + cnt]
        off_src += cnt
        off = xy * cap
        a_q[off:off + cnt] = (a_vals[seg] // 4).astype(np.int16)
        c_q[off:off + cnt] = (c_vals[seg] // 4).astype(np.int16)
        mask[off:off + cnt] = 1.0
    a_w = np.concatenate([_wrap_idx(a_q[xy * cap:(xy + 1) * cap], cap)
                          for xy in range(NBUCK)], axis=1)
    c_w = np.concatenate([_wrap_idx(c_q[xy * cap:(xy + 1) * cap], cap)
                          for xy in range(NBUCK)], axis=1)
    ncol = NBUCK * cap // 128
    mask_t = mask.reshape(ncol, 128).T.copy()
    return a_w, c_w, mask_t


def _round_cap(x):
    return max(128, ((int(x) + 127) // 128) * 128)


def _prepare(prototypes, pairs, neg_c):
    prototypes = np.ascontiguousarray(prototypes, dtype=np.float32)
    pairs = np.asarray(pairs, dtype=np.int32)
    neg_c = np.asarray(neg_c, dtype=np.int32)

    # per-core shards and global bucket capacities
    shards = []
    max_pos, max_neg = 0, 0
    for k in range(NCORES):
        pk = pairs[k * PPC:(k + 1) * PPC]
        nk = neg_c[k * NPC:(k + 1) * NPC]
        a, b = pk[:, 0], pk[:, 1]
        na = np.repeat(a, K)
        kp = (a % 4) * 4 + (b % 4)
        kn = (na % 4) * 4 + (nk % 4)
        max_pos = max(max_pos, int(np.bincount(kp, minlength=NBUCK).max()))
        max_neg = max(max_neg, int(np.bincount(kn, minlength=NBUCK).max()))
        shards.append((a, b, na, nk))
    capp = _round_cap(max_pos)
    capn = _round_cap(max_neg)

    in_maps = []
    for k in range(NCORES):
        a, b, na, nk = shards[k]
        pa, pb, mp = _prep_stream(a, b, capp)
        ng_a, ng_c, mn = _prep_stream(na, nk, capn)
        in_maps.append({
            "prototypes": prototypes,
            "posa_i": pa, "posb_i": pb,
            "nega_i": ng_a, "negc_i": ng_c,
            "maskp": mp, "maskn": mn,
        })
    return capp, capn, in_maps


def kernel(prototypes, pairs, neg_c):
    from concourse.bass_utils import run_bass_kernel_spmd

    capp, capn, in_maps = _prepare(prototypes, pairs, neg_c)
    key = (capp, capn)
    if key not in _CACHE:
        _CACHE[key] = _build_program(capp, capn)
    nc = _CACHE[key]

    res = run_bass_kernel_spmd(nc, in_maps, core_ids=list(range(NCORES)))
    pos_sum = 0.0
    neg_sum = 0.0
    for k in range(NCORES):
        part = res.results[k]["partials"]
        pos_sum += float(part[:, 0].sum(dtype=np.float64))
        neg_sum += float(part[:, 1].sum(dtype=np.float64))
    loss = 0.5 * (pos_sum / P_TOT + neg_sum / (P_TOT * K))
    return np.float32(loss)



# revision 2
# speedup vs baseline: 2.2573x; 2.2573x over previous
"""EntailmentConeLoss on 8 Trainium2 NeuronCores — v2.

Data-parallel over pairs (8192 pos + 32768 neg per core), prototype table
replicated in bf16 (tolerance 2e-2; dots of 256-dim bf16 rows carry ~0.3%
error and the loss averages 327680 energies, so bf16 is safe).

Per core:
- Rows are fetched with gpsimd dma_gather(transpose=True) on a 4-row-strided
  bf16 table view (int16 quotient indices, residue per bucket). Each gathered
  row is 512B and lands TRANSPOSED: tile [128, 2, n] holds element 128*t+p of
  pair-column j at [p, t, j]. Pairs are bucket-sorted by (a%4, c%4) on the
  host; one gather call per bucket per role (64 calls/core).
- Elementwise a*c / a^2 / c^2 in bf16 (DVE tensor_tensor at 2x, ACT Square;
  c^2 alternates between the two for load balance).
- Reduction over D uses the TensorEngine: per 128-pair block, the product
  tile is the STATIONARY operand [128 d-half, 128 pairs] and a ones column is
  moving, so psum[:, blk] = per-pair dot, accumulated over the two d-halves.
  Pair j of the global column order lands at psum partition j%128, col j//128.
- f32 epilogue (octant-reduced arccos, arcsin series) identical to v1, on
  [128, nblocks] tiles; masked sums -> partials [128, 2], summed on host.
"""
import os
os.environ.setdefault("NEURON_RT_RESET_CORES", "1")

import numpy as np

C, D = 100000, 256
P_TOT, K = 65536, 4
NCORES = 8
PPC = P_TOT // NCORES          # pos pairs per core
NPC = PPC * K                  # neg pairs per core
NBUCK = 16
EPS = np.float32(1e-6)
BETA = np.float32(0.1)
MARGIN = np.float32(0.1)
QUEUES = int(os.environ.get("KQ", "4"))
TBL_FP8 = os.environ.get("KFP8", "0") == "1"
SQC_MODE = int(os.environ.get("KSQC", "3"))   # sqc on ACT every Nth (0=always ACT)
SINGLE_PACKET = os.environ.get("KSP", "0") == "1"

_CACHE = {}


def _build_program(capp, capn, loop_iters=1, stage=5):
    import concourse.bass as bass
    import concourse.bacc as bacc
    import concourse.mybir as mybir
    import concourse.tile as tile

    f32 = mybir.dt.float32
    bf16 = mybir.dt.bfloat16
    gdt = mybir.dt.float8e4 if TBL_FP8 else bf16
    i16 = mybir.dt.int16
    Alu = mybir.AluOpType
    Act = mybir.ActivationFunctionType

    NPOS = NBUCK * capp
    NNEG = NBUCK * capn
    NPB = NPOS // 128              # pos 128-pair blocks
    NNB = NNEG // 128
    NB = NPB + NNB

    nc = bacc.Bacc("TRN2", target_bir_lowering=False, num_devices=NCORES,
                   num_swdge_queues=4)
    tbl = nc.dram_tensor("tblbf", [C, D], gdt, kind="ExternalInput")
    posa_i = nc.dram_tensor("posa_i", [128, NPOS // 16], i16, kind="ExternalInput")
    posb_i = nc.dram_tensor("posb_i", [128, NPOS // 16], i16, kind="ExternalInput")
    nega_i = nc.dram_tensor("nega_i", [128, NNEG // 16], i16, kind="ExternalInput")
    negc_i = nc.dram_tensor("negc_i", [128, NNEG // 16], i16, kind="ExternalInput")
    maskp = nc.dram_tensor("maskp", [128, NPB], f32, kind="ExternalInput")
    maskn = nc.dram_tensor("maskn", [128, NNB], f32, kind="ExternalInput")
    partials = nc.dram_tensor("partials", [128, 4], f32, kind="ExternalOutput")

    HALF_PI = float(np.float32(np.pi / 2))

    with tile.TileContext(nc) as tc:
        with tc.tile_pool(name="io", bufs=1) as io, \
             tc.tile_pool(name="gat", bufs=2) as gat, \
             tc.tile_pool(name="ew", bufs=2) as ew, \
             tc.tile_pool(name="ps", bufs=1, space="PSUM") as ps, \
             tc.tile_pool(name="tmp", bufs=1) as tmp:

            posa_t = io.tile([128, NPOS // 16], i16)
            posb_t = io.tile([128, NPOS // 16], i16)
            nega_t = io.tile([128, NNEG // 16], i16)
            negc_t = io.tile([128, NNEG // 16], i16)
            maskp_t = io.tile([128, NPB], f32)
            maskn_t = io.tile([128, NNB], f32)
            nc.sync.dma_start(out=posa_t[:], in_=posa_i[:])
            nc.sync.dma_start(out=posb_t[:], in_=posb_i[:])
            nc.sync.dma_start(out=nega_t[:], in_=nega_i[:])
            nc.sync.dma_start(out=negc_t[:], in_=negc_i[:])
            nc.sync.dma_start(out=maskp_t[:], in_=maskp[:])
            nc.sync.dma_start(out=maskn_t[:], in_=maskn[:])

            ones_t = io.tile([128, 1], bf16)
            nc.vector.memset(ones_t[:], 1.0)

            NH0 = NNB // 2
            NH1 = NNB - NH0
            sb_p = io.tile([128, 3 * NPB], f32)
            sb_n0 = io.tile([128, 3 * NH0], f32)
            sb_n1 = io.tile([128, 3 * NH1], f32)
            out_t = io.tile([128, 4], f32)
            nc.vector.memset(out_t[:], 0.0)

            tview = tbl[:].rearrange("(q r) d -> q r d", r=4)

            qrr = [0]
            sqc_rr = [0]

            def loop_body(_i=None):
                # psum group tiles: [pp cols | cc cols | pc cols], one bank each
                def psgroup(tag, ncols):
                    return ps.tile([128, 3 * ncols], f32, tag=tag, name=tag)

                def stream(a_idx_t, c_idx_t, cap, grp, blkbase, tagp, buckets):
                    icap16 = cap // 16
                    gncol = grp.shape[1] // 3
                    for xy in buckets:
                        ra, rc = xy // 4, xy % 4
                        A = gat.tile([128, 2, cap], gdt, tag=tagp + "ga",
                                     name=tagp + "ga")
                        Cc = gat.tile([128, 2, cap], gdt, tag=tagp + "gc",
                                      name=tagp + "gc")
                        nc.gpsimd.dma_gather(
                            A[:], tview[:, ra, :],
                            a_idx_t[:, xy * icap16:(xy + 1) * icap16],
                            cap, cap, D, elem_step=4 * D, transpose=True,
                            single_packet=SINGLE_PACKET, queue_num=qrr[0] % QUEUES)
                        nc.gpsimd.dma_gather(
                            Cc[:], tview[:, rc, :],
                            c_idx_t[:, xy * icap16:(xy + 1) * icap16],
                            cap, cap, D, elem_step=4 * D, transpose=True,
                            single_packet=SINGLE_PACKET, queue_num=(qrr[0] + 1) % QUEUES)
                        qrr[0] += 2
                        if stage < 1:
                            continue
                        Af = A[:].rearrange("p a b -> p (a b)")
                        Cf = Cc[:].rearrange("p a b -> p (a b)")
                        prod = ew.tile([128, 2, cap], bf16, tag=tagp + "pr",
                                       name=tagp + "pr")
                        sqa = ew.tile([128, 2, cap], bf16, tag=tagp + "sa",
                                      name=tagp + "sa")
                        sqc = ew.tile([128, 2, cap], bf16, tag=tagp + "sc",
                                      name=tagp + "sc")
                        nc.vector.tensor_tensor(
                            out=prod[:].rearrange("p a b -> p (a b)"),
                            in0=Af, in1=Cf, op=Alu.mult)
                        nc.scalar.activation(
                            out=sqa[:].rearrange("p a b -> p (a b)"),
                            in_=Af, func=Act.Square)
                        # alternate engine for c^2 to balance DVE vs ACT
                        sqc_rr[0] += 1
                        if SQC_MODE == 0 or sqc_rr[0] % SQC_MODE == 0:
                            nc.scalar.activation(
                                out=sqc[:].rearrange("p a b -> p (a b)"),
                                in_=Cf, func=Act.Square)
                        else:
                            nc.vector.tensor_tensor(
                                out=sqc[:].rearrange("p a b -> p (a b)"),
                                in0=Cf, in1=Cf, op=Alu.mult)
                        if stage < 2:
                            continue
                        nblk = cap // 128
                        for t in range(nblk):
                            col = xy * nblk + t - blkbase
                            for qi, tl in ((0, sqa), (1, sqc), (2, prod)):
                                pcol = qi * gncol + col
                                nc.tensor.matmul(
                                    grp[:, pcol:pcol + 1],
                                    tl[:, 0, t * 128:(t + 1) * 128],
                                    ones_t[:, 0:1],
                                    start=True, stop=False)
                                nc.tensor.matmul(
                                    grp[:, pcol:pcol + 1],
                                    tl[:, 1, t * 128:(t + 1) * 128],
                                    ones_t[:, 0:1],
                                    start=False, stop=True)

                # ---------------- epilogue (wide f32 ops) ----------------
                # stage 3: through cos; stage 4: + arccos; stage >=5: full
                def group_epilogue(grp, sbt, mask_ap, ncol, is_neg, out_col):
                    if stage < 2:
                        return
                    nc.vector.tensor_copy(sbt[:], grp[:])
                    if stage >= 3:
                        epilogue(sbt[:, 0:ncol], sbt[:, ncol:2 * ncol],
                                 sbt[:, 2 * ncol:3 * ncol], mask_ap, ncol,
                                 is_neg, out_col)

                def epilogue(pp_b, cc_b, pc_b, mask_t, ncol, is_neg, out_col):
                    T = lambda nm: tmp.tile([128, ncol], f32, tag="ep" + nm,
                                            name="ep" + nm)
                    ppcc = T("ppcc")
                    nc.vector.tensor_tensor(out=ppcc[:], in0=cc_b, in1=pp_b, op=Alu.add)
                    t2 = T("t2")
                    nc.vector.tensor_scalar(out=t2[:], in0=pc_b, scalar1=-2.0,
                                            scalar2=None, op0=Alu.mult)
                    dd = T("dd")
                    nc.vector.tensor_tensor(out=dd[:], in0=ppcc[:], in1=t2[:], op=Alu.add)
                    # near-duplicate guard: dd is rounding junk when c≈p (the
                    # three dots come from different engine paths); force
                    # cos=0 (ang=pi/2) like the reference's eps-denominator.
                    dupf = T("dupf")
                    nc.vector.tensor_scalar(out=dupf[:], in0=ppcc[:], scalar1=2e-3,
                                            scalar2=None, op0=Alu.mult)
                    nc.vector.tensor_tensor(out=dupf[:], in0=dd[:], in1=dupf[:], op=Alu.is_lt)
                    nc.vector.tensor_scalar(out=dupf[:], in0=dupf[:], scalar1=-1.0,
                                            scalar2=1.0, op0=Alu.mult, op1=Alu.add)
                    nc.vector.tensor_scalar(out=dd[:], in0=dd[:], scalar1=0.0,
                                            scalar2=None, op0=Alu.max)
                    g = T("g")
                    nc.vector.tensor_tensor(out=g[:], in0=pp_b, in1=dd[:], op=Alu.mult)
                    # s = sqrt(g) + one Newton step; +1e-30 keeps g=0 finite
                    nc.vector.tensor_scalar(out=g[:], in0=g[:], scalar1=1e-30,
                                            scalar2=None, op0=Alu.add)
                    s0 = T("s0")
                    nc.scalar.activation(out=s0[:], in_=g[:], func=Act.Sqrt)
                    r = T("r")
                    nc.vector.reciprocal(r[:], s0[:])
                    s1 = T("s1")
                    nc.vector.tensor_tensor(out=s1[:], in0=g[:], in1=r[:], op=Alu.mult)
                    nc.vector.tensor_tensor(out=s1[:], in0=s1[:], in1=s0[:], op=Alu.add)
                    den = T("den")
                    nc.vector.tensor_scalar(out=den[:], in0=s1[:], scalar1=float(EPS),
                                            scalar2=None, op0=Alu.add)
                    rden = T("rden")
                    nc.vector.reciprocal(rden[:], den[:])
                    num = T("num")
                    nc.vector.tensor_tensor(out=num[:], in0=pc_b, in1=pp_b, op=Alu.subtract)
                    cos = T("cos")
                    nc.vector.tensor_tensor(out=cos[:], in0=num[:], in1=rden[:], op=Alu.mult)
                    nc.vector.tensor_scalar(out=cos[:], in0=cos[:], scalar1=2.0,
                                            scalar2=float(-(1.0 - 1e-6)), op0=Alu.mult,
                                            op1=Alu.max)
                    nc.vector.tensor_scalar(out=cos[:], in0=cos[:], scalar1=float(1.0 - 1e-6),
                                            scalar2=None, op0=Alu.min)
                    nc.vector.tensor_tensor(out=cos[:], in0=cos[:], in1=dupf[:], op=Alu.mult)
                    if stage == 3:
                        nc.vector.tensor_tensor(out=cos[:], in0=cos[:], in1=mask_t, op=Alu.mult)
                        nc.vector.tensor_reduce(
                            out=out_t[:, out_col:out_col + 1], in_=cos[:],
                            axis=mybir.AxisListType.X, op=Alu.add)
                        return
                    # ang = arccos(cos) via octant-reduced arctan
                    q = T("q")
                    nc.vector.tensor_tensor(out=q[:], in0=cos[:], in1=cos[:], op=Alu.mult)
                    nc.vector.tensor_scalar(out=q[:], in0=q[:], scalar1=-1.0,
                                            scalar2=1.0, op0=Alu.mult, op1=Alu.add)
                    q0 = T("q0")
                    nc.scalar.activation(out=q0[:], in_=q[:], func=Act.Sqrt)
                    rq = T("rq")
                    nc.vector.reciprocal(rq[:], q0[:])
                    sq = T("sq")
                    nc.vector.tensor_tensor(out=sq[:], in0=q[:], in1=rq[:], op=Alu.mult)
                    nc.vector.tensor_tensor(out=sq[:], in0=sq[:], in1=q0[:], op=Alu.add)
                    nc.vector.tensor_scalar(out=sq[:], in0=sq[:], scalar1=0.5,
                                            scalar2=None, op0=Alu.mult)
                    abst = T("abst")
                    nc.vector.tensor_scalar(out=abst[:], in0=cos[:], scalar1=-1.0,
                                            scalar2=None, op0=Alu.mult)
                    nc.vector.tensor_tensor(out=abst[:], in0=abst[:], in1=cos[:], op=Alu.max)
                    u = T("u")
                    nc.vector.tensor_tensor(out=u[:], in0=abst[:], in1=sq[:], op=Alu.min)
                    v = T("v")
                    nc.vector.tensor_tensor(out=v[:], in0=abst[:], in1=sq[:], op=Alu.max)
                    rv = T("rv")
                    nc.vector.reciprocal(rv[:], v[:])
                    rr = T("rr")
                    nc.vector.tensor_tensor(out=rr[:], in0=u[:], in1=rv[:], op=Alu.mult)
                    at = T("at")
                    nc.scalar.activation(out=at[:], in_=rr[:], func=Act.Arctan)
                    pg = T("pg")
                    nc.vector.tensor_scalar(out=pg[:], in0=cos[:], scalar1=0.0,
                                            scalar2=None, op0=Alu.is_gt)
                    ng = T("ng")
                    nc.vector.tensor_scalar(out=ng[:], in0=cos[:], scalar1=0.0,
                                            scalar2=None, op0=Alu.is_lt)
                    sgn = T("sgn")
                    nc.vector.tensor_tensor(out=sgn[:], in0=pg[:], in1=ng[:], op=Alu.subtract)
                    big = T("big")
                    nc.vector.tensor_tensor(out=big[:], in0=abst[:], in1=sq[:], op=Alu.is_gt)
                    c1 = T("c1")
                    nc.vector.tensor_scalar(out=c1[:], in0=big[:], scalar1=2.0,
                                            scalar2=-1.0, op0=Alu.mult, op1=Alu.add)
                    nc.vector.tensor_tensor(out=c1[:], in0=c1[:], in1=sgn[:], op=Alu.mult)
                    c0 = T("c0")
                    nc.vector.tensor_tensor(out=c0[:], in0=big[:], in1=ng[:], op=Alu.mult)
                    nc.vector.tensor_scalar(out=c0[:], in0=c0[:], scalar1=float(np.pi),
                                            scalar2=None, op0=Alu.mult)
                    c0b = T("c0b")
                    nc.vector.tensor_scalar(out=c0b[:], in0=big[:], scalar1=-HALF_PI,
                                            scalar2=HALF_PI, op0=Alu.mult, op1=Alu.add)
                    nc.vector.tensor_tensor(out=c0[:], in0=c0[:], in1=c0b[:], op=Alu.add)
                    ang = T("ang")
                    nc.vector.tensor_tensor(out=ang[:], in0=c1[:], in1=at[:], op=Alu.mult)
                    nc.vector.tensor_tensor(out=ang[:], in0=ang[:], in1=c0[:], op=Alu.add)
                    if stage == 4:
                        nc.vector.tensor_tensor(out=ang[:], in0=ang[:], in1=mask_t, op=Alu.mult)
                        nc.vector.tensor_reduce(
                            out=out_t[:, out_col:out_col + 1], in_=ang[:],
                            axis=mybir.AxisListType.X, op=Alu.add)
                        return
                    # aperture = asin(0.1 / (sqrt(pp) + eps)), small-angle series
                    sp0 = T("sp0")
                    nc.scalar.activation(out=sp0[:], in_=pp_b, func=Act.Sqrt)
                    rp = T("rp")
                    nc.vector.reciprocal(rp[:], sp0[:])
                    sp1 = T("sp1")
                    nc.vector.tensor_tensor(out=sp1[:], in0=pp_b, in1=rp[:], op=Alu.mult)
                    nc.vector.tensor_tensor(out=sp1[:], in0=sp1[:], in1=sp0[:], op=Alu.add)
                    nc.vector.tensor_scalar(out=sp1[:], in0=sp1[:], scalar1=0.5,
                                            scalar2=float(EPS), op0=Alu.mult, op1=Alu.add)
                    rsp = T("rsp")
                    nc.vector.reciprocal(rsp[:], sp1[:])
                    y = T("y")
                    nc.vector.tensor_scalar(out=y[:], in0=rsp[:], scalar1=float(BETA),
                                            scalar2=0.0, op0=Alu.mult, op1=Alu.max)
                    nc.vector.tensor_scalar(out=y[:], in0=y[:], scalar1=float(1.0 - 1e-6),
                                            scalar2=None, op0=Alu.min)
                    y2 = T("y2")
                    nc.vector.tensor_tensor(out=y2[:], in0=y[:], in1=y[:], op=Alu.mult)
                    y3 = T("y3")
                    nc.vector.tensor_tensor(out=y3[:], in0=y2[:], in1=y[:], op=Alu.mult)
                    ap = T("ap")
                    nc.vector.tensor_scalar(out=ap[:], in0=y3[:], scalar1=float(1.0 / 6.0),
                                            scalar2=None, op0=Alu.mult)
                    nc.vector.tensor_tensor(out=ap[:], in0=ap[:], in1=y[:], op=Alu.add)
                    e = T("e")
                    nc.vector.tensor_tensor(out=e[:], in0=ang[:], in1=ap[:], op=Alu.subtract)
                    nc.vector.tensor_scalar(out=e[:], in0=e[:], scalar1=0.0,
                                            scalar2=None, op0=Alu.max)
                    if is_neg:
                        nc.vector.tensor_scalar(out=e[:], in0=e[:], scalar1=-1.0,
                                                scalar2=float(MARGIN), op0=Alu.mult,
                                                op1=Alu.add)
                        nc.vector.tensor_scalar(out=e[:], in0=e[:], scalar1=0.0,
                                                scalar2=None, op0=Alu.max)
                    nc.vector.tensor_tensor(out=e[:], in0=e[:], in1=mask_t, op=Alu.mult)
                    nc.vector.tensor_reduce(
                        out=out_t[:, out_col:out_col + 1], in_=e[:],
                        axis=mybir.AxisListType.X, op=Alu.add)

                grp_p = psgroup("gp", NPB)
                grp_n0 = psgroup("gn0", NH0)
                grp_n1 = psgroup("gn1", NH1)
                stream(posa_t, posb_t, capp, grp_p, 0, "p", range(NBUCK))
                group_epilogue(grp_p, sb_p, maskp_t[:], NPB, False, 0)
                stream(nega_t, negc_t, capn, grp_n0, 0, "n", range(NBUCK // 2))
                group_epilogue(grp_n0, sb_n0, maskn_t[:, 0:NH0], NH0, True, 1)
                stream(nega_t, negc_t, capn, grp_n1, (NBUCK // 2) * (capn // 128),
                       "n", range(NBUCK // 2, NBUCK))
                group_epilogue(grp_n1, sb_n1, maskn_t[:, NH0:NNB], NH1, True, 2)
                if stage < 3:
                    nc.vector.tensor_reduce(out=out_t[:, 0:1], in_=maskp_t[:],
                                            axis=mybir.AxisListType.X, op=Alu.add)
                    nc.vector.tensor_reduce(out=out_t[:, 1:2], in_=maskn_t[:],
                                            axis=mybir.AxisListType.X, op=Alu.add)
                    nc.vector.tensor_reduce(out=out_t[:, 2:3], in_=maskn_t[:],
                                            axis=mybir.AxisListType.X, op=Alu.add)
                    nc.vector.tensor_reduce(out=out_t[:, 3:4], in_=maskp_t[:],
                                            axis=mybir.AxisListType.X, op=Alu.add)
                nc.sync.dma_start(out=partials[:], in_=out_t[:])

            if loop_iters > 1:
                with tc.For_i(0, loop_iters, 1):
                    loop_body()
            else:
                loop_body()

    nc.compile()
    return nc


def _wrap_idx(q):
    """[n] int16 -> [128, n//16] wrapped+replicated gather-index layout."""
    w = q.reshape(-1, 16).T
    return np.tile(w, (8, 1))


def _prep_stream(a_vals, c_vals, cap):
    """Bucket (a, c) pairs by (a%4, c%4), pad buckets to cap columns.

    Returns int16 quotient idx tiles [128, 16*cap//16] for each role and the
    validity mask [128, 16*cap//128] in the distributed (partition=col%128,
    block=col//128) layout."""
    key = (a_vals % 4) * 4 + (c_vals % 4)
    order = np.argsort(key, kind="stable")
    counts = np.bincount(key, minlength=NBUCK)
    assert counts.max() <= cap, (counts.max(), cap)
    a_q = np.zeros(NBUCK * cap, np.int16)
    c_q = np.zeros(NBUCK * cap, np.int16)
    mask = np.zeros(NBUCK * cap, np.float32)
    off_src = 0
    for xy in range(NBUCK):
        cnt = int(counts[xy])
        seg = order[off_src:off_src + cnt]
        off_src += cnt
        off = xy * cap
        a_q[off:off + cnt] = (a_vals[seg] // 4).astype(np.int16)
        c_q[off:off + cnt] = (c_vals[seg] // 4).astype(np.int16)
        mask[off:off + cnt] = 1.0
    a_w = np.concatenate([_wrap_idx(a_q[xy * cap:(xy + 1) * cap])
                          for xy in range(NBUCK)], axis=1)
    c_w = np.concatenate([_wrap_idx(c_q[xy * cap:(xy + 1) * cap])
                          for xy in range(NBUCK)], axis=1)
    nblk = NBUCK * cap // 128
    mask_t = mask.reshape(nblk, 128).T.copy()
    return a_w, c_w, mask_t


def _round_cap(x):
    return max(128, ((int(x) + 127) // 128) * 128)


def _prepare(prototypes, pairs, neg_c):
    import ml_dtypes

    prototypes = np.ascontiguousarray(prototypes, dtype=np.float32)
    tblbf = prototypes.astype(ml_dtypes.float8_e4m3 if TBL_FP8 else ml_dtypes.bfloat16)
    pairs = np.asarray(pairs, dtype=np.int32)
    neg_c = np.asarray(neg_c, dtype=np.int32)

    shards = []
    max_pos, max_neg = 0, 0
    for k in range(NCORES):
        pk = pairs[k * PPC:(k + 1) * PPC]
        nk = neg_c[k * NPC:(k + 1) * NPC]
        a, b = pk[:, 0], pk[:, 1]
        na = np.repeat(a, K)
        kp = (a % 4) * 4 + (b % 4)
        kn = (na % 4) * 4 + (nk % 4)
        max_pos = max(max_pos, int(np.bincount(kp, minlength=NBUCK).max()))
        max_neg = max(max_neg, int(np.bincount(kn, minlength=NBUCK).max()))
        shards.append((a, b, na, nk))
    capp = _round_cap(max_pos)
    capn = _round_cap(max_neg)

    in_maps = []
    for k in range(NCORES):
        a, b, na, nk = shards[k]
        pa, pb, mp = _prep_stream(a, b, capp)
        ng_a, ng_c, mn = _prep_stream(na, nk, capn)
        in_maps.append({
            "tblbf": tblbf,
            "posa_i": pa, "posb_i": pb,
            "nega_i": ng_a, "negc_i": ng_c,
            "maskp": mp, "maskn": mn,
        })
    return capp, capn, in_maps


def kernel(prototypes, pairs, neg_c):
    from concourse.bass_utils import run_bass_kernel_spmd

    capp, capn, in_maps = _prepare(prototypes, pairs, neg_c)
    key = (capp, capn)
    if key not in _CACHE:
        _CACHE[key] = _build_program(capp, capn)
    nc = _CACHE[key]

    res = run_bass_kernel_spmd(nc, in_maps, core_ids=list(range(NCORES)))
    pos_sum = 0.0
    neg_sum = 0.0
    for k in range(NCORES):
        part = res.results[k]["partials"]
        pos_sum += float(part[:, 0].sum(dtype=np.float64))
        neg_sum += float(part[:, 1].sum(dtype=np.float64))
        neg_sum += float(part[:, 2].sum(dtype=np.float64))
    loss = 0.5 * (pos_sum / P_TOT + neg_sum / (P_TOT * K))
    return np.float32(loss)
